# revision 5
# baseline (speedup 1.0000x reference)
"""Trainium2 Bass kernel for nn_BertWordPair (ragged RoPE pair scores).

Strategy (v2)
-------------
Inputs: qw, kw (B=8, S=768, H=4, D=256) fp32; token_index, thread_id (S,) int32.
Output: (B, S, S, H) fp32 where each (row-block, col-block) pair of the 6x128
thread-block grid uses one of three RoPE sign regimes:
    pp: rope(q,+pos) . rope(k,+pos)
    np: rope(q,-pos) . rope(k,+pos)   (0 < ti_r < ti_c)
    pn: rope(q,+pos) . rope(k,-pos)   (ti_c > 0, ti_r > ti_c)

Host precomputes the +rotated variants q+, k+ de-interleaved (pair-index,
token) fp16 with the token blocks PERMUTED so the rotation-run blocks land
first; one dialogue per core (batch data-parallel over 8 cores).  Both q-
and k- are derived on-device from q+/k+ by DVE fp16 rotations
(x- = R(-2theta) x+) against a small [c2|s2|c2] table (token pattern repeats
per 128-block so one 384-col table serves every block, broadcast across
heads with a stride-0 AP dim).  Device: 288 fp16 matmuls into 2-bank PSUM
pair tiles, one fused f32->f16 evacuation copy per pair (ACT/Pool), and
contiguous half-row output DMAs of the fp16 [S, NB*H*BLK] output, which the
host transposes back to (S, S, H) and upcasts.  DMA total ~7.9MB/core
(~22us @360GB/s) with rotations/matmuls/evac hidden underneath.
"""

import os

import numpy as np

ROPE_BASE = 10000.0
B, S, H, D = 8, 768, 4, 256
HALF = D // 2  # 128
BLK = 128
NB = S // BLK  # 6
N_CORES = 8

_prog_cache = {}


def _host_rotations(qw, kw, token_index):
    """Return (qp_u, qp_v), (kp_u, kp_v): +rotated even/odd planes, fp32,
    each (B, S, H, HALF)."""
    inv_freq = np.power(
        np.float32(ROPE_BASE),
        (np.arange(HALF, dtype=np.float32) * np.float32(-2.0 / D)),
    )
    pos = token_index.astype(np.float32)
    theta = pos[:, None] * inv_freq[None, :]  # (S, HALF)
    cos = np.cos(theta)[None, :, None, :]
    sin = np.sin(theta)[None, :, None, :]
    out = []
    for x in (qw, kw):
        u = x[..., 0::2]
        v = x[..., 1::2]
        out.append((u * cos - v * sin, v * cos + u * sin))
    return out


def _to_device_layout(u, v, perm):
    """(B,S,H,HALF) u/v planes -> (B, H, 2, HALF, S) fp16, tokens in perm
    block order."""
    cols = np.concatenate([np.arange(b * BLK, (b + 1) * BLK) for b in perm])
    u = u[:, cols]
    v = v[:, cols]
    arr = np.stack([u, v], axis=2)  # (B,T,2,H,HALF)
    arr = np.transpose(arr, (0, 3, 2, 4, 1))  # (B,H,2,HALF,T)
    return np.ascontiguousarray(arr.astype(np.float16))


def _regime_map(thread_id):
    tid = np.asarray(thread_id)
    if tid.shape[0] != S:
        return None, False
    blocks = tid.reshape(NB, BLK)
    if not np.all(blocks == blocks[:, :1]):
        return None, False
    tvals = blocks[:, 0]
    regimes = []
    for i in range(NB):
        row = []
        for j in range(NB):
            ti_r, ti_c = tvals[i], tvals[j]
            if ti_r > 0 and ti_r < ti_c:
                row.append("np")
            elif ti_c > 0 and ti_r > ti_c:
                row.append("pn")
            else:
                row.append("pp")
        regimes.append(row)
    return regimes, True


def _schedule(regimes, R):
    """Static schedule: rotation emission order, PE row order, and per-row
    column orders, tuned for the DMA arrival order (rotation-run blocks
    first, then k+, then leftover blocks)."""
    need_qn = [i for i in range(NB) if any(r == "np" for r in regimes[i])]
    need_kn = sorted(
        {j for i in range(NB) for j in range(NB) if regimes[i][j] == "pn"}
    )
    # Interleave q/k rotations, weighted so early-consumed blocks come first.
    rot_order = []
    qq = [("q", b) for b in R if b in need_qn]
    kk = [("k", b) for b in R if b in need_kn]
    pattern = [0, 0, 1, 0, 0, 1, 1, 1]  # q,q,k,q,q,k,k,k for 4+4
    qi = ki = 0
    for p in pattern[: len(qq) + len(kk)]:
        if (p == 0 and qi < len(qq)) or ki >= len(kk):
            rot_order.append(qq[qi])
            qi += 1
        else:
            rot_order.append(kk[ki])
            ki += 1
    rot_order += qq[qi:] + kk[ki:]

    # PE row order: rotation rows that only need early qn first, then the
    # all-pp rows (gated on the late leftover-block DMA), then the rest.
    rows = list(range(NB))
    def row_key(i):
        if i in R[:2]:
            return (0, i)
        if all(r == "pp" for r in regimes[i]):
            return (1, i)
        return (2, i)
    row_order = sorted(rows, key=row_key)

    # Within a row: rotate the column order to start at the diagonal, so
    # late-gated columns (leftover blocks b0/b5, late kn blocks) land last.
    j_orders = {i: [(i + d) % NB for d in range(NB)] for i in range(NB)}
    return rot_order, row_order, j_orders


def _build_program(regimes, R):
    import concourse.bass as bass  # noqa: F401
    import concourse.tile as tile
    from concourse import bacc, mybir
    from concourse.bass import broadcast_tensor_aps

    f16 = mybir.dt.float16
    f32 = mybir.dt.float32

    nR = len(R)
    TR = nR * BLK
    perm = list(R) + [b for b in range(NB) if b not in R]
    pos_of = {b: i for i, b in enumerate(perm)}
    r_pos = {b: i for i, b in enumerate(R)}
    rot_order, row_order, j_orders = _schedule(regimes, R)

    nc = bacc.Bacc(None, target_bir_lowering=False)
    qp_d = nc.dram_tensor("qp", [H, 2, HALF, S], f16, kind="ExternalInput")
    kp_d = nc.dram_tensor("kp", [H, 2, HALF, S], f16, kind="ExternalInput")
    kt_d = nc.dram_tensor("kt", [HALF, 3 * BLK], f16, kind="ExternalInput")
    out_d = nc.dram_tensor("out", [S, NB * H * BLK], f16, kind="ExternalOutput")

    with tile.TileContext(nc) as tc:
        with (
            tc.tile_pool(name="inp", bufs=1) as inp,
            tc.tile_pool(name="psum", bufs=4, space="PSUM") as pp,
            tc.tile_pool(name="stage", bufs=3) as stp,
            tc.tile_pool(name="rtmp", bufs=4) as rtmp,
        ):
            qp_t = inp.tile([HALF, H * 2 * S], f16, tag="qp")
            kp_t = inp.tile([HALF, H * 2 * S], f16, tag="kp")
            qn_t = inp.tile([HALF, H * 2 * TR], f16, tag="qn")
            kn_t = inp.tile([HALF, H * 2 * TR], f16, tag="kn")
            kt_t = inp.tile([HALF, 3 * BLK], f16, tag="kt")

            qp_v = qp_t[:].rearrange("p (h c t) -> p h c t", h=H, c=2, t=S)
            kp_v = kp_t[:].rearrange("p (h c t) -> p h c t", h=H, c=2, t=S)
            qp_dv = qp_d[:].rearrange("h c p t -> p h c t")
            kp_dv = kp_d[:].rearrange("h c p t -> p h c t")

            # Input DMA stream: table, rotation-run q+ (2-block x chunk
            # pieces so rotations start early), then k+, then the leftover
            # (non-rotation) blocks of both.  512B+ descriptor rows always.
            nc.sync.dma_start(kt_t[:], kt_d[:])
            half_r = (nR + 1) // 2 * BLK
            for tsl in (slice(0, half_r), slice(half_r, TR)):
                if tsl.start >= tsl.stop:
                    continue
                for c in range(2):
                    nc.sync.dma_start(qp_v[:, :, c, tsl], qp_dv[:, :, c, tsl])
            for c in range(2):
                nc.sync.dma_start(
                    kp_v[:, :, c, 0:TR], kp_dv[:, :, c, 0:TR]
                )
            if TR < S:
                nc.sync.dma_start(
                    kp_v[:, :, :, TR:S], kp_dv[:, :, :, TR:S]
                )
                nc.sync.dma_start(
                    qp_v[:, :, :, TR:S], qp_dv[:, :, :, TR:S]
                )

            # On-device rotations: x- = R(-2theta) x+ per rotation block.
            #   x-_u = xu*c2 + xv*s2 ; x-_v = xv*c2 - xu*s2
            # fused over (h, c) with the table broadcast across heads:
            #   X = [xu|xv] * [c2|s2] -> x-_u = X.c0 + X.c1
            #   Y = [xu|xv] * [s2|c2] -> x-_v = Y.c1 - Y.c0
            tabA = kt_t[:, 0 : 2 * BLK].rearrange(
                "p (o c t) -> p o c t", o=1, c=2
            )
            tabB = kt_t[:, BLK : 3 * BLK].rearrange(
                "p (o c t) -> p o c t", o=1, c=2
            )
            qn_v = qn_t[:].rearrange("p (h c t) -> p h c t", h=H, c=2, t=TR)
            kn_v = kn_t[:].rearrange("p (h c t) -> p h c t", h=H, c=2, t=TR)

            def emit_rotation(which, blk):
                src_v = qp_v if which == "q" else kp_v
                dst_v = qn_v if which == "q" else kn_v
                pr = r_pos[blk]
                tsl = slice(pr * BLK, (pr + 1) * BLK)
                pepo = src_v[:, :, :, tsl]  # (p, h, 2, BLK)
                tx = rtmp.tile([HALF, H * 2 * BLK], f16, tag="tx")
                ty = rtmp.tile([HALF, H * 2 * BLK], f16, tag="ty")
                tx4 = tx[:].rearrange("p (h c t) -> p h c t", h=H, c=2, t=BLK)
                ty4 = ty[:].rearrange("p (h c t) -> p h c t", h=H, c=2, t=BLK)
                inA, tA = broadcast_tensor_aps(pepo, tabA)
                inB, tB = broadcast_tensor_aps(pepo, tabB)
                nc.vector.tensor_mul(tx4, inA, tA)
                nc.vector.tensor_mul(ty4, inB, tB)
                nc.vector.tensor_add(
                    dst_v[:, :, 0, tsl], tx4[:, :, 0], tx4[:, :, 1]
                )
                nc.vector.tensor_sub(
                    dst_v[:, :, 1, tsl], ty4[:, :, 1], ty4[:, :, 0]
                )

            for which, blk in rot_order:
                emit_rotation(which, blk)

            def lhs_slice(variant, h, c, blk):
                if variant == "p":
                    return qp_t[:, (h * 2 + c) * S + pos_of[blk] * BLK :][:, :BLK]
                return qn_t[:, (h * 2 + c) * TR + r_pos[blk] * BLK :][:, :BLK]

            def rhs_slice(variant, h, c, blk):
                if variant == "p":
                    return kp_t[:, (h * 2 + c) * S + pos_of[blk] * BLK :][:, :BLK]
                return kn_t[:, (h * 2 + c) * TR + r_pos[blk] * BLK :][:, :BLK]

            evac_cycle = ["act", "pool"]
            evac_n = 0
            for i in row_order:
                order = j_orders[i]
                stage = stp.tile([BLK, NB * H * BLK], f16, tag="stage")
                pairs = [
                    pp.tile([BLK, 2 * H * BLK], f32, tag="pair", name=f"pair{i}_{k}")
                    for k in range(NB // 2)
                ]

                def mm(pos, c):
                    j = order[pos]
                    reg = regimes[i][j]
                    qv = "n" if reg == "np" else "p"
                    kv = "n" if reg == "pn" else "p"
                    bank = pairs[pos // 2]
                    off = (pos % 2) * (H * BLK)
                    for h in range(H):
                        nc.tensor.matmul(
                            bank[:, off + h * BLK : off + (h + 1) * BLK],
                            lhs_slice(qv, h, c, i),
                            rhs_slice(kv, h, c, j),
                            start=(c == 0 and h == 0),
                            stop=(c == 1 and h == H - 1),
                        )

                # early group (first 4 columns) c0 then c1; late group after.
                for c in range(2):
                    for pos in range(4):
                        mm(pos, c)
                for c in range(2):
                    for pos in range(4, NB):
                        mm(pos, c)

                sent = [False] * NB

                def send(j, j2):
                    posrun = [order.index(jj) for jj in range(j, j2 + 1)]
                    p0 = min(posrun)
                    n = len(posrun)
                    nc.sync.dma_start(
                        out_d[
                            i * BLK : (i + 1) * BLK,
                            j * H * BLK : (j2 + 1) * H * BLK,
                        ],
                        stage[:, p0 * H * BLK : (p0 + n) * H * BLK],
                    )
                    for jj in range(j, j2 + 1):
                        sent[jj] = True

                def flush_runs(done_pos, final=False):
                    """DMA maximal contiguous dst-j runs fully evacuated."""
                    done_j = {order[p] for p in range(done_pos)}
                    j = 0
                    while j < NB:
                        if sent[j] or j not in done_j:
                            j += 1
                            continue
                        j2 = j
                        while j2 + 1 < NB and not sent[j2 + 1] and (j2 + 1) in done_j:
                            j2 += 1
                        # src positions must be contiguous too
                        posrun = [order.index(jj) for jj in range(j, j2 + 1)]
                        if posrun == list(range(min(posrun), max(posrun) + 1)):
                            send(j, j2)
                        elif final:
                            for jj in range(j, j2 + 1):
                                send(jj, jj)
                        j = j2 + 1

                for pi, bank in enumerate(pairs):
                    dst = stage[:, pi * 2 * H * BLK : (pi + 1) * 2 * H * BLK]
                    eng = evac_cycle[evac_n % len(evac_cycle)]
                    evac_n += 1
                    if eng == "act":
                        nc.scalar.copy(dst, bank[:])
                    elif eng == "pool":
                        nc.gpsimd.tensor_copy(dst, bank[:])
                    else:
                        nc.vector.tensor_copy(dst, bank[:])
                    flush_runs(2 * (pi + 1), final=(pi == len(pairs) - 1))
    nc.finalize()
    return nc


def _reference_fallback(qw, kw, token_index, thread_id):
    """Pure numpy fallback for unexpected structure."""
    inv_freq = np.power(
        np.float32(ROPE_BASE),
        (np.arange(HALF, dtype=np.float32) * np.float32(-2.0 / D)),
    )
    pos = token_index.astype(np.float32)
    theta = pos[:, None] * inv_freq[None, :]
    cos = np.cos(theta)[None, :, None, :]
    sin = np.sin(theta)[None, :, None, :]

    def rot(x, sgn):
        u = x[..., 0::2]
        v = x[..., 1::2]
        ru = u * cos - sgn * v * sin
        rv = v * cos + sgn * u * sin
        o = np.empty(x.shape, dtype=np.float32)
        o[..., 0::2] = ru
        o[..., 1::2] = rv
        return o

    q_p = rot(qw, 1.0)
    q_n = rot(qw, -1.0)
    k_p = rot(kw, 1.0)
    k_n = rot(kw, -1.0)
    s_pp = np.einsum("bmhd,bnhd->bmnh", q_p, k_p)
    s_np = np.einsum("bmhd,bnhd->bmnh", q_n, k_p)
    s_pn = np.einsum("bmhd,bnhd->bmnh", q_p, k_n)
    ti_r = thread_id[:, None]
    ti_c = thread_id[None, :]
    sx = ((ti_r > 0) & (ti_r < ti_c))[None, :, :, None]
    sy = ((ti_c > 0) & (ti_r > ti_c))[None, :, :, None]
    return np.where(sx, s_np, np.where(sy, s_pn, s_pp)).astype(np.float32)


def _plan(thread_id, token_index):
    """Validate structure; return (regimes, R) or None."""
    regimes, ok = _regime_map(thread_id)
    if not ok:
        return None
    ti = np.asarray(token_index).reshape(NB, BLK)
    if not np.all(ti == ti[:1]):
        return None
    qn_blocks = sorted(
        {i for i in range(NB) if any(r == "np" for r in regimes[i])}
    )
    kn_blocks = sorted(
        {j for i in range(NB) for j in range(NB) if regimes[i][j] == "pn"}
    )
    R = sorted(set(qn_blocks) | set(kn_blocks))
    if not R:
        R = [0]
    if R != list(range(R[0], R[0] + len(R))):
        return None
    return regimes, R


def kernel(qw, kw, token_index, thread_id):
    qw = np.asarray(qw, dtype=np.float32)
    kw = np.asarray(kw, dtype=np.float32)
    token_index = np.asarray(token_index)
    thread_id = np.asarray(thread_id)

    plan = None
    if (
        qw.shape == (B, S, H, D)
        and kw.shape == (B, S, H, D)
        and token_index.shape == (S,)
    ):
        plan = _plan(thread_id, token_index)
    if plan is None:
        return _reference_fallback(qw, kw, token_index, thread_id)
    regimes, R = plan
    perm = list(R) + [b for b in range(NB) if b not in R]

    (qp_u, qp_v), (kp_u, kp_v) = _host_rotations(qw, kw, token_index)
    qp_a = _to_device_layout(qp_u, qp_v, perm)
    kp_a = _to_device_layout(kp_u, kp_v, perm)

    inv_freq = np.power(
        np.float32(ROPE_BASE),
        (np.arange(HALF, dtype=np.float32) * np.float32(-2.0 / D)),
    )
    theta = token_index[: BLK].astype(np.float32)[:, None] * inv_freq[None, :]
    c2 = np.cos(2.0 * theta).T  # (HALF, BLK)
    s2 = np.sin(2.0 * theta).T
    kt_a = np.ascontiguousarray(
        np.concatenate([c2, s2, c2], axis=1).astype(np.float16)
    )

    key = (tuple(tuple(r) for r in regimes), tuple(R))
    if key not in _prog_cache:
        _prog_cache[key] = _build_program(regimes, R)
    nc = _prog_cache[key]

    from concourse.bass_utils import run_bass_kernel_spmd

    in_maps = [
        {"qp": qp_a[b], "kp": kp_a[b], "kt": kt_a} for b in range(B)
    ]
    trace = bool(int(os.environ.get("KERNEL_TRACE", "0")))
    res = None
    for attempt in range(3):
        try:
            res = run_bass_kernel_spmd(
                nc,
                in_maps,
                core_ids=list(range(N_CORES)),
                trace=trace,
            )
            break
        except Exception:
            if attempt == 2:
                raise
    if res.exec_time_ns is not None:
        print(f"HW exec time: {res.exec_time_ns} ns")
    if res.instructions_and_trace is not None:
        print(f"trace: {res.instructions_and_trace[1]}")

    out = np.stack([res.results[b]["out"] for b in range(B)], axis=0)
    # (B, S, NB*H*BLK) fp16 -> (B, S, S, H) fp32
    out = out.reshape(B, S, NB, H, BLK).transpose(0, 1, 2, 4, 3)
    out = out.reshape(B, S, S, H).astype(np.float32)
    return out
